# revision 2
# baseline (speedup 1.0000x reference)
"""Trainium2 Bass kernel for ClassicalSelfAttention.

  out = softmax((x @ Wq) @ (x @ Wk)^T / sqrt(D)) @ x      x: [8192, 1024] f32

Sharding (8 NeuronCores): rows of x are sharded across cores; each core
projects its own row-shard to Q^T and K^T, the K^T shards are AllGathered
across cores (SDMA, overlaps compute), and each core runs a streaming
attention loop over 16 key-blocks of 512 keys: scores matmul -> fused
exp+rowsum on ScalarE -> PE transpose of the prob block -> PV matmul
accumulated in SBUF. The softmax division is folded into the final output
scale. 1/sqrt(1024) = 2^-5 is folded into Wq on the host (exact in fp32).
"""

import sys

import numpy as np

try:
    import concourse.bass as bass  # noqa: F401
except ImportError:  # pragma: no cover
    sys.path.insert(0, "/opt/trn_rl_repo")

import concourse.bacc as bacc
import concourse.mybir as mybir
import concourse.tile as tile
from concourse import bass_utils
from concourse.masks import make_identity

N_TOKENS = 8192
EMBED = 1024
NCORES = 8
M = N_TOKENS // NCORES  # rows per core (1024)
P = 128  # partitions
DC = EMBED // P  # contraction chunks (8)
NB = 512  # key-block width
NNB = N_TOKENS // NB  # key blocks (16)
MB = M // P  # query row-blocks per core (8)
VC = NB // P  # value chunks per key block (4)
FP32 = mybir.dt.float32
EXP = mybir.ActivationFunctionType.Exp
ADD = mybir.AluOpType.add
AXX = mybir.AxisListType.X


def _build():
    nc = bacc.Bacc(
        "TRN2", target_bir_lowering=False, debug=False, num_devices=NCORES
    )
    xt_shard = nc.dram_tensor("xt_shard", [EMBED, M], FP32, kind="ExternalInput").ap()
    x_full = nc.dram_tensor(
        "x_full", [N_TOKENS, EMBED], FP32, kind="ExternalInput"
    ).ap()
    wq_d = nc.dram_tensor("wq", [EMBED, EMBED], FP32, kind="ExternalInput").ap()
    wk_d = nc.dram_tensor("wk", [EMBED, EMBED], FP32, kind="ExternalInput").ap()
    out_d = nc.dram_tensor("out", [M, EMBED], FP32, kind="ExternalOutput").ap()

    wq_r = wq_d.rearrange("(a p) d -> a p d", p=P)  # [DC, P, EMBED]
    wk_r = wk_d.rearrange("(a p) d -> a p d", p=P)
    xt_r = xt_shard.rearrange("(a p) m -> a p m", p=P)  # [DC, P, M]
    xv_r = x_full.rearrange("(t p) d -> t p d", p=P)  # [64, P, EMBED]
    out_r = out_d.rearrange("(t p) d -> t p d", p=P)  # [MB, P, EMBED]

    with tile.TileContext(nc) as tc:
        with (
            tc.tile_pool(name="persist", bufs=1) as pers,
            tc.tile_pool(name="persist_dram", bufs=1, space="DRAM") as pdram,
        ):
            ident = pers.tile([P, P], FP32)
            make_identity(nc, ident[:])
            # Q^T resident for the whole kernel: qt[p, b*M + m] = Qt[b*128+p, m]
            qt = pers.tile([P, DC * M], FP32)
            # fp32 PV accumulator per query block: [p, mb*EMBED + dv]
            out_acc = pers.tile([P, MB * EMBED], FP32)
            # row-sum partials: one column per (mb, nb)
            sums = pers.tile([P, MB * NNB], FP32)
            # K^T shard (AllGather input) and gathered K^T of all cores
            ktd = pdram.tile([DC, P, M], FP32)
            gkt = pdram.tile([NCORES * DC, P, M], FP32, addr_space="Shared")

            # ---- Phase A: project Q^T (own rows) and K^T shard, AllGather K^T
            with (
                tc.tile_pool(name="proj", bufs=1) as proj,
                tc.tile_pool(name="proj_ps", bufs=4, space="PSUM") as proj_ps,
            ):
                wq_sb = proj.tile([P, DC * EMBED], FP32)
                wk_sb = proj.tile([P, DC * EMBED], FP32)
                xt_sb = proj.tile([P, DC * M], FP32)
                ktsb = proj.tile([P, DC * M], FP32)
                for a in range(DC):
                    nc.sync.dma_start(
                        out=wk_sb[:, a * EMBED : (a + 1) * EMBED], in_=wk_r[a]
                    )
                    nc.sync.dma_start(
                        out=xt_sb[:, a * M : (a + 1) * M], in_=xt_r[a]
                    )
                    nc.sync.dma_start(
                        out=wq_sb[:, a * EMBED : (a + 1) * EMBED], in_=wq_r[a]
                    )
                # K^T first so its AllGather overlaps the Q^T projection.
                for w_sb, dst in ((wk_sb, ktsb), (wq_sb, qt)):
                    for b in range(DC):  # output dim chunk
                        for j in range(M // NB):  # row half
                            ps = proj_ps.tile([P, NB], FP32, tag="proj_ps")
                            for a in range(DC):  # contraction chunk
                                nc.tensor.matmul(
                                    ps[:],
                                    lhsT=w_sb[:, a * EMBED + b * P : a * EMBED + (b + 1) * P],
                                    rhs=xt_sb[:, a * M + j * NB : a * M + (j + 1) * NB],
                                    start=(a == 0),
                                    stop=(a == DC - 1),
                                )
                            nc.vector.tensor_copy(
                                out=dst[:, b * M + j * NB : b * M + (j + 1) * NB],
                                in_=ps[:],
                            )
                    if dst is ktsb:
                        for b in range(DC):
                            nc.sync.dma_start(
                                out=ktd[b], in_=ktsb[:, b * M : (b + 1) * M]
                            )
                        nc.gpsimd.collective_compute(
                            "AllGather",
                            mybir.AluOpType.bypass,
                            replica_groups=[list(range(NCORES))],
                            ins=[ktd.opt()],
                            outs=[gkt.opt()],
                        )

            # ---- Phase B: streaming attention over key blocks
            with (
                tc.tile_pool(name="kv", bufs=2) as kvp,
                tc.tile_pool(name="pb", bufs=3) as pbp,
                tc.tile_pool(name="ps_s", bufs=2, space="PSUM") as ps_sp,
                tc.tile_pool(name="ps_t", bufs=2, space="PSUM") as ps_tp,
                tc.tile_pool(name="ps_o", bufs=2, space="PSUM") as ps_op,
            ):
                for nb in range(NNB):
                    rank, half = nb // (M // NB), nb % (M // NB)
                    ktile = kvp.tile([P, DC * NB], FP32, tag="ktile")
                    for b in range(DC):
                        nc.sync.dma_start(
                            out=ktile[:, b * NB : (b + 1) * NB],
                            in_=gkt[rank * DC + b, :, half * NB : (half + 1) * NB],
                        )
                    vtile = kvp.tile([P, VC * EMBED], FP32, tag="vtile")
                    for c in range(VC):
                        nc.sync.dma_start(
                            out=vtile[:, c * EMBED : (c + 1) * EMBED],
                            in_=xv_r[nb * VC + c],
                        )
                    for mb in range(MB):
                        ps_s = ps_sp.tile([P, NB], FP32, tag="ps_s")
                        for b in range(DC):
                            nc.tensor.matmul(
                                ps_s[:],
                                lhsT=qt[:, b * M + mb * P : b * M + (mb + 1) * P],
                                rhs=ktile[:, b * NB : (b + 1) * NB],
                                start=(b == 0),
                                stop=(b == DC - 1),
                            )
                        pblk = pbp.tile([P, NB], FP32, tag="pblk")
                        col = mb * NNB + nb
                        nc.scalar.activation(
                            out=pblk[:],
                            in_=ps_s[:],
                            func=EXP,
                            accum_out=sums[:, col : col + 1],
                        )
                        ps_t = ps_tp.tile([P, NB], FP32, tag="ps_t")
                        for t in range(VC):
                            nc.tensor.transpose(
                                out=ps_t[:, t * P : (t + 1) * P],
                                in_=pblk[:, t * P : (t + 1) * P],
                                identity=ident[:],
                            )
                        ptb = pbp.tile([P, NB], FP32, tag="ptb")
                        nc.vector.tensor_copy(out=ptb[:], in_=ps_t[:])
                        for h in range(EMBED // NB):
                            ps_o = ps_op.tile([P, NB], FP32, tag="ps_o")
                            for t in range(VC):
                                nc.tensor.matmul(
                                    ps_o[:],
                                    lhsT=ptb[:, t * P : (t + 1) * P],
                                    rhs=vtile[:, t * EMBED + h * NB : t * EMBED + (h + 1) * NB],
                                    start=(t == 0),
                                    stop=(t == VC - 1),
                                )
                            dst = out_acc[:, mb * EMBED + h * NB : mb * EMBED + (h + 1) * NB]
                            if nb == 0:
                                nc.vector.tensor_copy(out=dst, in_=ps_o[:])
                            else:
                                nc.vector.tensor_tensor(
                                    out=dst, in0=dst, in1=ps_o[:], op=ADD
                                )

            # ---- Phase C: divide by softmax sum, write out
            with tc.tile_pool(name="fin", bufs=2) as fin:
                for mb in range(MB):
                    tot = fin.tile([P, 1], FP32, tag="tot")
                    nc.vector.reduce_sum(
                        out=tot[:], in_=sums[:, mb * NNB : (mb + 1) * NNB], axis=AXX
                    )
                    rtot = fin.tile([P, 1], FP32, tag="rtot")
                    nc.vector.reciprocal(out=rtot[:], in_=tot[:])
                    outf = fin.tile([P, EMBED], FP32, tag="outf")
                    nc.vector.tensor_scalar_mul(
                        outf[:], out_acc[:, mb * EMBED : (mb + 1) * EMBED], rtot[:]
                    )
                    nc.sync.dma_start(out=out_r[mb], in_=outf[:])

    nc.compile()
    return nc


_NC = None


def _get_nc():
    global _NC
    if _NC is None:
        _NC = _build()
    return _NC


def _run(x, rotation_params, entangle_params, **spmd_kwargs):
    x = np.ascontiguousarray(np.asarray(x, dtype=np.float32))
    wq = np.asarray(rotation_params, dtype=np.float32).reshape(EMBED, EMBED) * np.float32(
        1.0 / 32.0
    )
    wk = np.asarray(entangle_params, dtype=np.float32).reshape(EMBED, EMBED)
    xt = np.ascontiguousarray(x.T)
    in_maps = [
        {
            "xt_shard": np.ascontiguousarray(xt[:, i * M : (i + 1) * M]),
            "x_full": x,
            "wq": wq,
            "wk": wk,
        }
        for i in range(NCORES)
    ]
    res = bass_utils.run_bass_kernel_spmd(
        _get_nc(), in_maps, core_ids=list(range(NCORES)), **spmd_kwargs
    )
    out = np.concatenate([res.results[i]["out"] for i in range(NCORES)], axis=0)
    return out, res


def kernel(x, rotation_params, entangle_params):
    out, _ = _run(x, rotation_params, entangle_params)
    return out


# revision 5
# speedup vs baseline: 2.8974x; 2.8974x over previous
"""Trainium2 Bass kernel for ClassicalSelfAttention.

  out = softmax((x @ Wq) @ (x @ Wk)^T / sqrt(D)) @ x      x: [8192, 1024] f32

Sharding (8 NeuronCores): rows of x are sharded across cores; each core
projects its own row-shard to Q^T and K^T, the K^T shards are AllGathered
across cores (SDMA, overlaps compute), and each core runs a streaming
attention loop over 16 key-blocks of 512 keys: scores matmul -> fused
exp+rowsum on ScalarE -> PE transpose of the prob block -> PV matmul
accumulated in SBUF. The softmax division is folded into the final output
scale. 1/sqrt(1024) = 2^-5 is folded into Wq on the host (exact in fp32).
"""

import sys

import numpy as np

try:
    import concourse.bass as bass  # noqa: F401
except ImportError:  # pragma: no cover
    sys.path.insert(0, "/opt/trn_rl_repo")

import concourse.bacc as bacc
import concourse.mybir as mybir
import concourse.tile as tile
from concourse import bass_utils
from concourse.masks import make_identity

N_TOKENS = 8192
EMBED = 1024
NCORES = 8
M = N_TOKENS // NCORES  # rows per core (1024)
P = 128  # partitions
DC = EMBED // P  # contraction chunks (8)
NB = 512  # key-block width
NNB = N_TOKENS // NB  # key blocks (16)
MB = M // P  # query row-blocks per core (8)
VC = NB // P  # value chunks per key block (4)
FP32 = mybir.dt.float32
R32 = mybir.dt.float32r
EXP = mybir.ActivationFunctionType.Exp
ADD = mybir.AluOpType.add
AXX = mybir.AxisListType.X


def _build():
    nc = bacc.Bacc(
        "TRN2", target_bir_lowering=False, debug=False, num_devices=NCORES
    )
    xt_shard = nc.dram_tensor("xt_shard", [EMBED, M], R32, kind="ExternalInput").ap()
    x_full = nc.dram_tensor(
        "x_full", [N_TOKENS, EMBED], R32, kind="ExternalInput"
    ).ap()
    wq_d = nc.dram_tensor("wq", [EMBED, EMBED], R32, kind="ExternalInput").ap()
    wk_d = nc.dram_tensor("wk", [EMBED, EMBED], R32, kind="ExternalInput").ap()
    out_d = nc.dram_tensor("out", [M, EMBED], FP32, kind="ExternalOutput").ap()

    wq_r = wq_d.rearrange("(a p) d -> a p d", p=P)  # [DC, P, EMBED]
    wk_r = wk_d.rearrange("(a p) d -> a p d", p=P)
    xt_r = xt_shard.rearrange("(a p) m -> a p m", p=P)  # [DC, P, M]
    xv_r = x_full.rearrange("(t p) d -> t p d", p=P)  # [64, P, EMBED]
    out_r = out_d.rearrange("(t p) d -> t p d", p=P)  # [MB, P, EMBED]

    with tile.TileContext(nc) as tc:
        with (
            tc.tile_pool(name="persist", bufs=1) as pers,
            tc.tile_pool(name="persist_dram", bufs=1, space="DRAM") as pdram,
        ):
            ident = pers.tile([P, P], FP32)
            make_identity(nc, ident[:])
            # Q^T resident for the whole kernel: qt[p, b*M + m] = Qt[b*128+p, m]
            qt = pers.tile([P, DC * M], R32)
            # fp32 PV accumulator per query block: [p, mb*EMBED + dv]
            out_acc = pers.tile([P, MB * EMBED], FP32)
            # row-sum partials: one column per (mb, nb)
            sums = pers.tile([P, MB * NNB], FP32)
            # K^T shard (AllGather input) and gathered K^T of all cores
            ktd = pdram.tile([DC, P, M], R32)
            gkt = pdram.tile([NCORES * DC, P, M], R32, addr_space="Shared")

            # ---- Phase A: project Q^T (own rows) and K^T shard, AllGather K^T
            with (
                tc.tile_pool(name="proj", bufs=1) as proj,
                tc.tile_pool(name="proj_ps", bufs=4, space="PSUM") as proj_ps,
            ):
                wq_sb = proj.tile([P, DC * EMBED], R32)
                wk_sb = proj.tile([P, DC * EMBED], R32)
                xt_sb = proj.tile([P, DC * M], R32)
                ktsb = proj.tile([P, DC * M], R32)
                for a in range(DC):
                    nc.sync.dma_start(
                        out=wk_sb[:, a * EMBED : (a + 1) * EMBED], in_=wk_r[a]
                    )
                    nc.sync.dma_start(
                        out=xt_sb[:, a * M : (a + 1) * M], in_=xt_r[a]
                    )
                    nc.sync.dma_start(
                        out=wq_sb[:, a * EMBED : (a + 1) * EMBED], in_=wq_r[a]
                    )
                # K^T first so its AllGather overlaps the Q^T projection.
                for w_sb, dst in ((wk_sb, ktsb), (wq_sb, qt)):
                    for b in range(DC):  # output dim chunk
                        for j in range(M // NB):  # row half
                            ps = proj_ps.tile([P, NB], FP32, tag="proj_ps")
                            for a in range(DC):  # contraction chunk
                                nc.tensor.matmul(
                                    ps[:],
                                    lhsT=w_sb[:, a * EMBED + b * P : a * EMBED + (b + 1) * P],
                                    rhs=xt_sb[:, a * M + j * NB : a * M + (j + 1) * NB],
                                    start=(a == 0),
                                    stop=(a == DC - 1),
                                )
                            nc.vector.tensor_copy(
                                out=dst[:, b * M + j * NB : b * M + (j + 1) * NB],
                                in_=ps[:],
                            )
                    if dst is ktsb:
                        for b in range(DC):
                            nc.sync.dma_start(
                                out=ktd[b], in_=ktsb[:, b * M : (b + 1) * M]
                            )
                        nc.gpsimd.collective_compute(
                            "AllGather",
                            mybir.AluOpType.bypass,
                            replica_groups=[list(range(NCORES))],
                            ins=[ktd.opt()],
                            outs=[gkt.opt()],
                        )

            # ---- Phase B: streaming attention over key blocks
            with (
                tc.tile_pool(name="kv", bufs=2) as kvp,
                tc.tile_pool(name="pb", bufs=3) as pbp,
                tc.tile_pool(name="ps_s", bufs=2, space="PSUM") as ps_sp,
                tc.tile_pool(name="ps_t", bufs=2, space="PSUM") as ps_tp,
                tc.tile_pool(name="ps_o", bufs=2, space="PSUM") as ps_op,
            ):
                for nb in range(NNB):
                    rank, half = nb // (M // NB), nb % (M // NB)
                    ktile = kvp.tile([P, DC * NB], R32, tag="ktile")
                    for b in range(DC):
                        nc.sync.dma_start(
                            out=ktile[:, b * NB : (b + 1) * NB],
                            in_=gkt[rank * DC + b, :, half * NB : (half + 1) * NB],
                        )
                    vtile = kvp.tile([P, VC * EMBED], R32, tag="vtile")
                    for c in range(VC):
                        nc.sync.dma_start(
                            out=vtile[:, c * EMBED : (c + 1) * EMBED],
                            in_=xv_r[nb * VC + c],
                        )
                    for mb in range(MB):
                        ps_s = ps_sp.tile([P, NB], FP32, tag="ps_s")
                        for b in range(DC):
                            nc.tensor.matmul(
                                ps_s[:],
                                lhsT=qt[:, b * M + mb * P : b * M + (mb + 1) * P],
                                rhs=ktile[:, b * NB : (b + 1) * NB],
                                start=(b == 0),
                                stop=(b == DC - 1),
                            )
                        pblk = pbp.tile([P, NB], FP32, tag="pblk")
                        col = mb * NNB + nb
                        nc.scalar.activation(
                            out=pblk[:],
                            in_=ps_s[:],
                            func=EXP,
                            accum_out=sums[:, col : col + 1],
                        )
                        ps_t = ps_tp.tile([P, NB], FP32, tag="ps_t")
                        for t in range(VC):
                            nc.tensor.transpose(
                                out=ps_t[:, t * P : (t + 1) * P],
                                in_=pblk[:, t * P : (t + 1) * P],
                                identity=ident[:],
                            )
                        ptb = pbp.tile([P, NB], R32, tag="ptb")
                        nc.vector.tensor_copy(out=ptb[:], in_=ps_t[:])
                        for h in range(EMBED // NB):
                            ps_o = ps_op.tile([P, NB], FP32, tag="ps_o")
                            for t in range(VC):
                                nc.tensor.matmul(
                                    ps_o[:],
                                    lhsT=ptb[:, t * P : (t + 1) * P],
                                    rhs=vtile[:, t * EMBED + h * NB : t * EMBED + (h + 1) * NB],
                                    start=(t == 0),
                                    stop=(t == VC - 1),
                                )
                            dst = out_acc[:, mb * EMBED + h * NB : mb * EMBED + (h + 1) * NB]
                            if nb == 0:
                                nc.vector.tensor_copy(out=dst, in_=ps_o[:])
                            else:
                                nc.vector.tensor_tensor(
                                    out=dst, in0=dst, in1=ps_o[:], op=ADD
                                )

            # ---- Phase C: divide by softmax sum, write out
            with tc.tile_pool(name="fin", bufs=2) as fin:
                for mb in range(MB):
                    tot = fin.tile([P, 1], FP32, tag="tot")
                    nc.vector.reduce_sum(
                        out=tot[:], in_=sums[:, mb * NNB : (mb + 1) * NNB], axis=AXX
                    )
                    rtot = fin.tile([P, 1], FP32, tag="rtot")
                    nc.vector.reciprocal(out=rtot[:], in_=tot[:])
                    outf = fin.tile([P, EMBED], FP32, tag="outf")
                    nc.vector.tensor_scalar_mul(
                        outf[:], out_acc[:, mb * EMBED : (mb + 1) * EMBED], rtot[:]
                    )
                    nc.sync.dma_start(out=out_r[mb], in_=outf[:])

    nc.compile()
    return nc


_NC = None


def _get_nc():
    global _NC
    if _NC is None:
        _NC = _build()
    return _NC


def _run(x, rotation_params, entangle_params, **spmd_kwargs):
    x = np.ascontiguousarray(np.asarray(x, dtype=np.float32))
    wq = np.asarray(rotation_params, dtype=np.float32).reshape(EMBED, EMBED) * np.float32(
        1.0 / 32.0
    )
    wk = np.asarray(entangle_params, dtype=np.float32).reshape(EMBED, EMBED)
    xt = np.ascontiguousarray(x.T)
    in_maps = [
        {
            "xt_shard": np.ascontiguousarray(xt[:, i * M : (i + 1) * M]),
            "x_full": x,
            "wq": wq,
            "wk": wk,
        }
        for i in range(NCORES)
    ]
    res = bass_utils.run_bass_kernel_spmd(
        _get_nc(), in_maps, core_ids=list(range(NCORES)), **spmd_kwargs
    )
    out = np.concatenate([res.results[i]["out"] for i in range(NCORES)], axis=0)
    return out, res


def kernel(x, rotation_params, entangle_params):
    out, _ = _run(x, rotation_params, entangle_params)
    return out


# revision 6
# speedup vs baseline: 3.2339x; 1.1161x over previous
"""Trainium2 Bass kernel for ClassicalSelfAttention.

  out = softmax((x @ Wq) @ (x @ Wk)^T / sqrt(D)) @ x      x: [8192, 1024] f32

Sharding (8 NeuronCores): rows of x are sharded across cores; each core
projects its own row-shard to Q^T and K^T, the K^T shards are AllGathered
across cores (SDMA, overlaps compute), and each core runs a streaming
attention loop over 16 key-blocks of 512 keys: scores matmul -> fused
exp+rowsum on ScalarE -> PE transpose of the prob block -> PV matmul
accumulated in SBUF. The softmax division is folded into the final output
scale. 1/sqrt(1024) = 2^-5 is folded into Wq on the host (exact in fp32).

Matmul inputs are float32r (full PE rate, ~1e-4 matmul error, fp32 PSUM
accumulation). To hide the AllGather latency each core processes its OWN
key blocks first straight out of SBUF (plus its own V rows from a
per-core x_shard input); the remaining 14 key blocks are fetched in
rank-rotated order (rank + j) % 8 via partition-id-based dynamic DMA
offsets, so no core waits on the gather before doing useful work.
Softmax over key blocks is order-invariant, so the rotation is free.
"""

import sys

import numpy as np

try:
    import concourse.bass as bass  # noqa: F401
except ImportError:  # pragma: no cover
    sys.path.insert(0, "/opt/trn_rl_repo")

import concourse.bacc as bacc
import concourse.mybir as mybir
import concourse.tile as tile
from concourse import bass_utils
from concourse.bass import ds
from concourse.masks import make_identity

N_TOKENS = 8192
EMBED = 1024
NCORES = 8
M = N_TOKENS // NCORES  # rows per core (1024)
P = 128  # partitions
DC = EMBED // P  # contraction chunks (8)
NB = 512  # key-block width
NNB = N_TOKENS // NB  # key blocks (16)
MB = M // P  # query row-blocks per core (8)
VC = NB // P  # value chunks per key block (4)
HPR = M // NB  # key-block halves per rank (2)
FP32 = mybir.dt.float32
R32 = mybir.dt.float32r
EXP = mybir.ActivationFunctionType.Exp
ADD = mybir.AluOpType.add
AXX = mybir.AxisListType.X


def _build():
    nc = bacc.Bacc(
        "TRN2", target_bir_lowering=False, debug=False, num_devices=NCORES
    )
    xt_shard = nc.dram_tensor("xt_shard", [EMBED, M], R32, kind="ExternalInput").ap()
    x_shard = nc.dram_tensor("x_shard", [M, EMBED], R32, kind="ExternalInput").ap()
    x_full = nc.dram_tensor(
        "x_full", [N_TOKENS, EMBED], R32, kind="ExternalInput"
    ).ap()
    wq_d = nc.dram_tensor("wq", [EMBED, EMBED], R32, kind="ExternalInput").ap()
    wk_d = nc.dram_tensor("wk", [EMBED, EMBED], R32, kind="ExternalInput").ap()
    out_d = nc.dram_tensor("out", [M, EMBED], FP32, kind="ExternalOutput").ap()

    wq_r = wq_d.rearrange("(a p) d -> a p d", p=P)  # [DC, P, EMBED]
    wk_r = wk_d.rearrange("(a p) d -> a p d", p=P)
    xt_r = xt_shard.rearrange("(a p) m -> a p m", p=P)  # [DC, P, M]
    xs_r = x_shard.rearrange("(t p) d -> t p d", p=P)  # [M//P, P, EMBED]
    xv_r = x_full.rearrange("(t p) d -> t p d", p=P)  # [64, P, EMBED]
    out_r = out_d.rearrange("(t p) d -> t p d", p=P)  # [MB, P, EMBED]

    with tile.TileContext(nc) as tc:
        with (
            tc.tile_pool(name="persist", bufs=1) as pers,
            tc.tile_pool(name="persist_dram", bufs=1, space="DRAM") as pdram,
        ):
            ident = pers.tile([P, P], FP32)
            make_identity(nc, ident[:])
            # Q^T resident for the whole kernel: qt[p, b*M + m] = Qt[b*128+p, m]
            qt = pers.tile([P, DC * M], R32)
            # own K^T shard, kept resident: ktsb[p, b*M + n] = Kt_own[b*128+p, n]
            ktsb = pers.tile([P, DC * M], R32)
            # fp32 PV accumulator per query block: [p, mb*EMBED + dv]
            out_acc = pers.tile([P, MB * EMBED], FP32)
            # row-sum partials: one column per (mb, local key block)
            sums = pers.tile([P, MB * NNB], FP32)
            # K^T shard (AllGather input) and gathered K^T of all cores
            ktd = pdram.tile([DC, P, M], R32)
            gkt = pdram.tile([NCORES * DC, P, M], R32, addr_space="Shared")

            rank = nc.gpsimd.partition_id()

            # ---- Phase A: project Q^T (own rows) and K^T shard, AllGather K^T
            with (
                tc.tile_pool(name="proj", bufs=1) as proj,
                tc.tile_pool(name="proj_ps", bufs=4, space="PSUM") as proj_ps,
            ):
                wq_sb = proj.tile([P, DC * EMBED], R32)
                wk_sb = proj.tile([P, DC * EMBED], R32)
                xt_sb = proj.tile([P, DC * M], R32)
                for a in range(DC):
                    nc.sync.dma_start(
                        out=wk_sb[:, a * EMBED : (a + 1) * EMBED], in_=wk_r[a]
                    )
                    nc.sync.dma_start(
                        out=xt_sb[:, a * M : (a + 1) * M], in_=xt_r[a]
                    )
                    nc.sync.dma_start(
                        out=wq_sb[:, a * EMBED : (a + 1) * EMBED], in_=wq_r[a]
                    )
                # K^T first so its AllGather overlaps the Q^T projection.
                for w_sb, dst in ((wk_sb, ktsb), (wq_sb, qt)):
                    for b in range(DC):  # output dim chunk
                        for j in range(M // NB):  # row half
                            ps = proj_ps.tile([P, NB], FP32, tag="proj_ps")
                            for a in range(DC):  # contraction chunk
                                nc.tensor.matmul(
                                    ps[:],
                                    lhsT=w_sb[:, a * EMBED + b * P : a * EMBED + (b + 1) * P],
                                    rhs=xt_sb[:, a * M + j * NB : a * M + (j + 1) * NB],
                                    start=(a == 0),
                                    stop=(a == DC - 1),
                                )
                            nc.vector.tensor_copy(
                                out=dst[:, b * M + j * NB : b * M + (j + 1) * NB],
                                in_=ps[:],
                            )
                    if dst is ktsb:
                        for b in range(DC):
                            nc.sync.dma_start(
                                out=ktd[b], in_=ktsb[:, b * M : (b + 1) * M]
                            )
                        nc.gpsimd.collective_compute(
                            "AllGather",
                            mybir.AluOpType.bypass,
                            replica_groups=[list(range(NCORES))],
                            ins=[ktd.opt()],
                            outs=[gkt.opt()],
                        )

            # ---- Phase B: streaming attention over key blocks, own rank first
            with (
                tc.tile_pool(name="kv", bufs=2) as kvp,
                tc.tile_pool(name="pb", bufs=3) as pbp,
                tc.tile_pool(name="ps_s", bufs=2, space="PSUM") as ps_sp,
                tc.tile_pool(name="ps_t", bufs=2, space="PSUM") as ps_tp,
                tc.tile_pool(name="ps_o", bufs=2, space="PSUM") as ps_op,
            ):
                for nb in range(NNB):  # local processing order
                    j, half = nb // HPR, nb % HPR  # j = rank offset
                    vtile = kvp.tile([P, VC * EMBED], R32, tag="vtile")
                    if j == 0:
                        # own keys: K^T already in SBUF, V rows from x_shard
                        for c in range(VC):
                            nc.sync.dma_start(
                                out=vtile[:, c * EMBED : (c + 1) * EMBED],
                                in_=xs_r[half * VC + c],
                            )
                        k_sb, k_off = ktsb, half * NB

                        def k_slice(b):
                            return ktsb[:, b * M + k_off : b * M + k_off + NB]

                    else:
                        src = (rank + j) % NCORES
                        for c in range(VC):
                            nc.gpsimd.dma_start(
                                out=vtile[:, c * EMBED : (c + 1) * EMBED],
                                in_=xv_r[
                                    ds(src * (M // P) + half * VC + c, 1)
                                ].squeeze(0),
                            )
                        ktile = kvp.tile([P, DC * NB], R32, tag="ktile")
                        for b in range(DC):
                            nc.gpsimd.dma_start(
                                out=ktile[:, b * NB : (b + 1) * NB],
                                in_=gkt[
                                    ds(src * DC + b, 1),
                                    :,
                                    half * NB : (half + 1) * NB,
                                ].squeeze(0),
                            )

                        def k_slice(b, _kt=ktile):
                            return _kt[:, b * NB : (b + 1) * NB]

                    for mb in range(MB):
                        ps_s = ps_sp.tile([P, NB], FP32, tag="ps_s")
                        for b in range(DC):
                            nc.tensor.matmul(
                                ps_s[:],
                                lhsT=qt[:, b * M + mb * P : b * M + (mb + 1) * P],
                                rhs=k_slice(b),
                                start=(b == 0),
                                stop=(b == DC - 1),
                            )
                        pblk = pbp.tile([P, NB], FP32, tag="pblk")
                        col = mb * NNB + nb
                        nc.scalar.activation(
                            out=pblk[:],
                            in_=ps_s[:],
                            func=EXP,
                            accum_out=sums[:, col : col + 1],
                        )
                        ps_t = ps_tp.tile([P, NB], FP32, tag="ps_t")
                        for t in range(VC):
                            nc.tensor.transpose(
                                out=ps_t[:, t * P : (t + 1) * P],
                                in_=pblk[:, t * P : (t + 1) * P],
                                identity=ident[:],
                            )
                        ptb = pbp.tile([P, NB], R32, tag="ptb")
                        nc.vector.tensor_copy(out=ptb[:], in_=ps_t[:])
                        for h in range(EMBED // NB):
                            ps_o = ps_op.tile([P, NB], FP32, tag="ps_o")
                            for t in range(VC):
                                nc.tensor.matmul(
                                    ps_o[:],
                                    lhsT=ptb[:, t * P : (t + 1) * P],
                                    rhs=vtile[:, t * EMBED + h * NB : t * EMBED + (h + 1) * NB],
                                    start=(t == 0),
                                    stop=(t == VC - 1),
                                )
                            dst = out_acc[:, mb * EMBED + h * NB : mb * EMBED + (h + 1) * NB]
                            if nb == 0:
                                nc.vector.tensor_copy(out=dst, in_=ps_o[:])
                            else:
                                nc.vector.tensor_tensor(
                                    out=dst, in0=dst, in1=ps_o[:], op=ADD
                                )

            # ---- Phase C: divide by softmax sum, write out
            with tc.tile_pool(name="fin", bufs=2) as fin:
                for mb in range(MB):
                    tot = fin.tile([P, 1], FP32, tag="tot")
                    nc.vector.reduce_sum(
                        out=tot[:], in_=sums[:, mb * NNB : (mb + 1) * NNB], axis=AXX
                    )
                    rtot = fin.tile([P, 1], FP32, tag="rtot")
                    nc.vector.reciprocal(out=rtot[:], in_=tot[:])
                    outf = fin.tile([P, EMBED], FP32, tag="outf")
                    nc.vector.tensor_scalar_mul(
                        outf[:], out_acc[:, mb * EMBED : (mb + 1) * EMBED], rtot[:]
                    )
                    nc.sync.dma_start(out=out_r[mb], in_=outf[:])

    nc.compile()
    return nc


_NC = None


def _get_nc():
    global _NC
    if _NC is None:
        _NC = _build()
    return _NC


def _run(x, rotation_params, entangle_params, **spmd_kwargs):
    x = np.ascontiguousarray(np.asarray(x, dtype=np.float32))
    wq = np.asarray(rotation_params, dtype=np.float32).reshape(EMBED, EMBED) * np.float32(
        1.0 / 32.0
    )
    wk = np.asarray(entangle_params, dtype=np.float32).reshape(EMBED, EMBED)
    xt = np.ascontiguousarray(x.T)
    in_maps = [
        {
            "xt_shard": np.ascontiguousarray(xt[:, i * M : (i + 1) * M]),
            "x_shard": np.ascontiguousarray(x[i * M : (i + 1) * M]),
            "x_full": x,
            "wq": wq,
            "wk": wk,
        }
        for i in range(NCORES)
    ]
    res = bass_utils.run_bass_kernel_spmd(
        _get_nc(), in_maps, core_ids=list(range(NCORES)), **spmd_kwargs
    )
    out = np.concatenate([res.results[i]["out"] for i in range(NCORES)], axis=0)
    return out, res


def kernel(x, rotation_params, entangle_params):
    out, _ = _run(x, rotation_params, entangle_params)
    return out


# revision 7
# speedup vs baseline: 3.6291x; 1.1222x over previous
"""Trainium2 Bass kernel for ClassicalSelfAttention.

  out = softmax((x @ Wq) @ (x @ Wk)^T / sqrt(D)) @ x      x: [8192, 1024] f32

Sharding (8 NeuronCores): rows of x are sharded across cores; each core
projects its own row-shard to Q^T and K^T, the K^T shards are AllGathered
across cores (SDMA, overlaps compute), and each core runs a streaming
attention loop over 16 key-blocks of 512 keys: scores matmul -> fused
exp+rowsum on ScalarE -> PE transpose of the prob block -> PV matmul
accumulated in SBUF. The softmax division is folded into the final output
scale. 1/sqrt(1024) = 2^-5 is folded into Wq on the host (exact in fp32).

Matmul inputs are float32r (full PE rate, ~1e-4 matmul error, fp32 PSUM
accumulation). To hide the AllGather latency each core processes its OWN
key blocks first straight out of SBUF (plus its own V rows from a
per-core x_shard input); the remaining 14 key blocks are fetched in
rank-rotated order (rank + j) % 8 via partition-id-based dynamic DMA
offsets, so no core waits on the gather before doing useful work.
Softmax over key blocks is order-invariant, so the rotation is free.
"""

import sys

import numpy as np

try:
    import concourse.bass as bass  # noqa: F401
except ImportError:  # pragma: no cover
    sys.path.insert(0, "/opt/trn_rl_repo")

import concourse.bacc as bacc
import concourse.mybir as mybir
import concourse.tile as tile
from concourse import bass_utils
from concourse.bass import ds
from concourse.masks import make_identity

N_TOKENS = 8192
EMBED = 1024
NCORES = 8
M = N_TOKENS // NCORES  # rows per core (1024)
P = 128  # partitions
DC = EMBED // P  # contraction chunks (8)
NB = 512  # key-block width
NNB = N_TOKENS // NB  # key blocks (16)
MB = M // P  # query row-blocks per core (8)
VC = NB // P  # value chunks per key block (4)
HPR = M // NB  # key-block halves per rank (2)
FP32 = mybir.dt.float32
R32 = mybir.dt.float32r
BF16 = mybir.dt.bfloat16
EXP = mybir.ActivationFunctionType.Exp
ADD = mybir.AluOpType.add
AXX = mybir.AxisListType.X


def _build():
    nc = bacc.Bacc(
        "TRN2", target_bir_lowering=False, debug=False, num_devices=NCORES
    )
    xt_shard = nc.dram_tensor("xt_shard", [EMBED, M], R32, kind="ExternalInput").ap()
    x_shard = nc.dram_tensor("x_shard", [M, EMBED], BF16, kind="ExternalInput").ap()
    x_full = nc.dram_tensor(
        "x_full", [N_TOKENS, EMBED], BF16, kind="ExternalInput"
    ).ap()
    wq_d = nc.dram_tensor("wq", [EMBED, EMBED], R32, kind="ExternalInput").ap()
    wk_d = nc.dram_tensor("wk", [EMBED, EMBED], R32, kind="ExternalInput").ap()
    out_d = nc.dram_tensor("out", [M, EMBED], FP32, kind="ExternalOutput").ap()

    wq_r = wq_d.rearrange("(a p) d -> a p d", p=P)  # [DC, P, EMBED]
    wk_r = wk_d.rearrange("(a p) d -> a p d", p=P)
    xt_r = xt_shard.rearrange("(a p) m -> a p m", p=P)  # [DC, P, M]
    xs_r = x_shard.rearrange("(t p) d -> t p d", p=P)  # [M//P, P, EMBED]
    xv_r = x_full.rearrange("(t p) d -> t p d", p=P)  # [64, P, EMBED]
    out_r = out_d.rearrange("(t p) d -> t p d", p=P)  # [MB, P, EMBED]

    with tile.TileContext(nc) as tc:
        with (
            tc.tile_pool(name="persist", bufs=1) as pers,
            tc.tile_pool(name="persist_dram", bufs=1, space="DRAM") as pdram,
        ):
            ident = pers.tile([P, P], BF16)
            make_identity(nc, ident[:])
            # Q^T resident for the whole kernel: qt[p, b*M + m] = Qt[b*128+p, m]
            qt = pers.tile([P, DC * M], BF16)
            # own K^T shard, kept resident: ktsb[p, b*M + n] = Kt_own[b*128+p, n]
            ktsb = pers.tile([P, DC * M], BF16)
            # fp32 PV accumulator per query block: [p, mb*EMBED + dv]
            out_acc = pers.tile([P, MB * EMBED], FP32)
            # row-sum partials: one column per (mb, local key block)
            sums = pers.tile([P, MB * NNB], FP32)
            # K^T shard (AllGather input) and gathered K^T of all cores
            ktd = pdram.tile([DC, P, M], BF16)
            gkt = pdram.tile([NCORES * DC, P, M], BF16, addr_space="Shared")

            rank = nc.gpsimd.partition_id()

            # ---- Phase A: project Q^T (own rows) and K^T shard, AllGather K^T
            with (
                tc.tile_pool(name="proj", bufs=1) as proj,
                tc.tile_pool(name="proj_ps", bufs=4, space="PSUM") as proj_ps,
            ):
                wq_sb = proj.tile([P, DC * EMBED], R32)
                wk_sb = proj.tile([P, DC * EMBED], R32)
                xt_sb = proj.tile([P, DC * M], R32)
                for a in range(DC):
                    nc.sync.dma_start(
                        out=wk_sb[:, a * EMBED : (a + 1) * EMBED], in_=wk_r[a]
                    )
                    nc.sync.dma_start(
                        out=xt_sb[:, a * M : (a + 1) * M], in_=xt_r[a]
                    )
                    nc.sync.dma_start(
                        out=wq_sb[:, a * EMBED : (a + 1) * EMBED], in_=wq_r[a]
                    )
                # K^T first so its AllGather overlaps the Q^T projection.
                for w_sb, dst in ((wk_sb, ktsb), (wq_sb, qt)):
                    for b in range(DC):  # output dim chunk
                        for j in range(M // NB):  # row half
                            ps = proj_ps.tile([P, NB], FP32, tag="proj_ps")
                            for a in range(DC):  # contraction chunk
                                nc.tensor.matmul(
                                    ps[:],
                                    lhsT=w_sb[:, a * EMBED + b * P : a * EMBED + (b + 1) * P],
                                    rhs=xt_sb[:, a * M + j * NB : a * M + (j + 1) * NB],
                                    start=(a == 0),
                                    stop=(a == DC - 1),
                                )
                            nc.vector.tensor_copy(
                                out=dst[:, b * M + j * NB : b * M + (j + 1) * NB],
                                in_=ps[:],
                            )
                    if dst is ktsb:
                        for b in range(DC):
                            nc.sync.dma_start(
                                out=ktd[b], in_=ktsb[:, b * M : (b + 1) * M]
                            )
                        nc.gpsimd.collective_compute(
                            "AllGather",
                            mybir.AluOpType.bypass,
                            replica_groups=[list(range(NCORES))],
                            ins=[ktd.opt()],
                            outs=[gkt.opt()],
                        )

            # ---- Phase B: streaming attention over key blocks, own rank first
            with (
                tc.tile_pool(name="kv", bufs=2) as kvp,
                tc.tile_pool(name="pb", bufs=3) as pbp,
                tc.tile_pool(name="ps_s", bufs=2, space="PSUM") as ps_sp,
                tc.tile_pool(name="ps_t", bufs=2, space="PSUM") as ps_tp,
                tc.tile_pool(name="ps_o", bufs=2, space="PSUM") as ps_op,
            ):
                for nb in range(NNB):  # local processing order
                    j, half = nb // HPR, nb % HPR  # j = rank offset
                    vtile = kvp.tile([P, VC * EMBED], BF16, tag="vtile")
                    if j == 0:
                        # own keys: K^T already in SBUF, V rows from x_shard
                        for c in range(VC):
                            nc.sync.dma_start(
                                out=vtile[:, c * EMBED : (c + 1) * EMBED],
                                in_=xs_r[half * VC + c],
                            )
                        k_sb, k_off = ktsb, half * NB

                        def k_slice(b):
                            return ktsb[:, b * M + k_off : b * M + k_off + NB]

                    else:
                        src = (rank + j) % NCORES
                        for c in range(VC):
                            nc.gpsimd.dma_start(
                                out=vtile[:, c * EMBED : (c + 1) * EMBED],
                                in_=xv_r[
                                    ds(src * (M // P) + half * VC + c, 1)
                                ].squeeze(0),
                            )
                        ktile = kvp.tile([P, DC * NB], BF16, tag="ktile")
                        for b in range(DC):
                            nc.gpsimd.dma_start(
                                out=ktile[:, b * NB : (b + 1) * NB],
                                in_=gkt[
                                    ds(src * DC + b, 1),
                                    :,
                                    half * NB : (half + 1) * NB,
                                ].squeeze(0),
                            )

                        def k_slice(b, _kt=ktile):
                            return _kt[:, b * NB : (b + 1) * NB]

                    for mb in range(MB):
                        ps_s = ps_sp.tile([P, NB], FP32, tag="ps_s")
                        for b in range(DC):
                            nc.tensor.matmul(
                                ps_s[:],
                                lhsT=qt[:, b * M + mb * P : b * M + (mb + 1) * P],
                                rhs=k_slice(b),
                                start=(b == 0),
                                stop=(b == DC - 1),
                            )
                        pblk = pbp.tile([P, NB], BF16, tag="pblk")
                        col = mb * NNB + nb
                        nc.scalar.activation(
                            out=pblk[:],
                            in_=ps_s[:],
                            func=EXP,
                            accum_out=sums[:, col : col + 1],
                        )
                        ps_t = ps_tp.tile([P, NB], BF16, tag="ps_t")
                        for t in range(VC):
                            nc.tensor.transpose(
                                out=ps_t[:, t * P : (t + 1) * P],
                                in_=pblk[:, t * P : (t + 1) * P],
                                identity=ident[:],
                            )
                        ptb = pbp.tile([P, NB], BF16, tag="ptb")
                        nc.vector.tensor_copy(out=ptb[:], in_=ps_t[:])
                        for h in range(EMBED // NB):
                            ps_o = ps_op.tile([P, NB], FP32, tag="ps_o")
                            for t in range(VC):
                                nc.tensor.matmul(
                                    ps_o[:],
                                    lhsT=ptb[:, t * P : (t + 1) * P],
                                    rhs=vtile[:, t * EMBED + h * NB : t * EMBED + (h + 1) * NB],
                                    start=(t == 0),
                                    stop=(t == VC - 1),
                                )
                            dst = out_acc[:, mb * EMBED + h * NB : mb * EMBED + (h + 1) * NB]
                            if nb == 0:
                                nc.vector.tensor_copy(out=dst, in_=ps_o[:])
                            else:
                                nc.vector.tensor_tensor(
                                    out=dst, in0=dst, in1=ps_o[:], op=ADD
                                )

            # ---- Phase C: divide by softmax sum, write out
            with tc.tile_pool(name="fin", bufs=2) as fin:
                for mb in range(MB):
                    tot = fin.tile([P, 1], FP32, tag="tot")
                    nc.vector.reduce_sum(
                        out=tot[:], in_=sums[:, mb * NNB : (mb + 1) * NNB], axis=AXX
                    )
                    rtot = fin.tile([P, 1], FP32, tag="rtot")
                    nc.vector.reciprocal(out=rtot[:], in_=tot[:])
                    outf = fin.tile([P, EMBED], FP32, tag="outf")
                    nc.vector.tensor_scalar_mul(
                        outf[:], out_acc[:, mb * EMBED : (mb + 1) * EMBED], rtot[:]
                    )
                    nc.sync.dma_start(out=out_r[mb], in_=outf[:])

    nc.compile()
    return nc


_NC = None


def _get_nc():
    global _NC
    if _NC is None:
        _NC = _build()
    return _NC


def _run(x, rotation_params, entangle_params, **spmd_kwargs):
    x = np.ascontiguousarray(np.asarray(x, dtype=np.float32))
    wq = np.asarray(rotation_params, dtype=np.float32).reshape(EMBED, EMBED) * np.float32(
        1.0 / 32.0
    )
    wk = np.asarray(entangle_params, dtype=np.float32).reshape(EMBED, EMBED)
    xt = np.ascontiguousarray(x.T)
    import ml_dtypes

    x_bf = x.astype(ml_dtypes.bfloat16)
    in_maps = [
        {
            "xt_shard": np.ascontiguousarray(xt[:, i * M : (i + 1) * M]),
            "x_shard": np.ascontiguousarray(x_bf[i * M : (i + 1) * M]),
            "x_full": x_bf,
            "wq": wq,
            "wk": wk,
        }
        for i in range(NCORES)
    ]
    res = bass_utils.run_bass_kernel_spmd(
        _get_nc(), in_maps, core_ids=list(range(NCORES)), **spmd_kwargs
    )
    out = np.concatenate([res.results[i]["out"] for i in range(NCORES)], axis=0)
    return out, res


def kernel(x, rotation_params, entangle_params):
    out, _ = _run(x, rotation_params, entangle_params)
    return out
